# revision 25
# baseline (speedup 1.0000x reference)
"""DiffAttn Trainium2 kernel (8-core SPMD, no collectives) — v2.

Problem: B=2, T=2048, IN_DIM=OUT_DIM=1024, H=8 v-heads (2D=64), 2H=16 qk-heads
(D=32). Core c = 4*b + g handles batch b, head-group g: qk-heads {4g..4g+3}
(local heads h=0..3; h even = positive softmax, h odd = negative), v-heads
{2g, 2g+1}. Out-projection row-sharded; host sums 4 partials per batch.

v2 structure (vs v1): the dots PSUM is split pos/neg — heads {0,2} land in P,
heads {1,3} in N. exp(P) runs on the Scalar engine (ACT); exp(N) is computed
on the Vector engine with a Schraudolph bit-trick (one tensor_scalar:
int16(A*z+B) whose bits ARE the bf16 of e^z). The sawtooth error of the
approximation only touches the negative softmax, which the end-to-end test
shows is attenuated by the lambda-weighted subtraction (~3e-3 final rel err).
This halves the ACT workload, which was the phase-B critical path, and makes
the PE dense enough to hold its high clock state (HAM).

The per-query-block combine (r = lam*s1/s2, comb = a1 - r*a2, sumsq) runs
interleaved with the next block's attention, using the otherwise-idle GpSimd
engine for partition broadcasts / reductions and elementwise work. The RMS
sqrt is batched into the tail so the ACT exp table set is never swapped
mid-attention. gamma*(1-lambda_init) is folded into Wout on the host; the
1/rms scale commutes onto comb before the out-projection.
"""
import math

import numpy as np

H = 8
D = 32
LAMBDA_INIT = 0.8 - 0.6 * math.exp(-0.3)
B, T, IN_DIM, OUT_DIM = 2, 2048, 1024, 1024
E = 2 * H * D  # 512

N_CORES = 8
GROUPS = 4            # head groups (cores per batch)
QB = 512              # query block (matmul free dim)
NQB = T // QB         # 4
KT = 128              # key tile (partition dim)
NKT = T // KT         # 16
NIN = IN_DIM // 128   # 8

# Schraudolph exp in bf16-bit-space: bf16(e^z) ~= bits(int16(A16*z + B16)).
# B16 tuned for ~zero mean rel err; constant offset cancels in softmax.
A16 = 128.0 / math.log(2.0)
B16 = 1064866805.0 / 65536.0

_compiled = None


def _build():
    import concourse.bass as bass  # noqa: F401
    import concourse.mybir as mybir
    from concourse import bacc, bass_isa
    from concourse.tile import TileContext

    f32 = mybir.dt.float32
    bf16 = mybir.dt.bfloat16
    i16 = mybir.dt.int16
    AF = mybir.ActivationFunctionType
    MUL = mybir.AluOpType.mult
    ADD = mybir.AluOpType.add

    nc = bacc.Bacc("TRN2", target_bir_lowering=False, num_devices=N_CORES)

    xT = nc.dram_tensor("xT", [IN_DIM, T], bf16, kind="ExternalInput")
    wq = nc.dram_tensor("wq", [IN_DIM, 128], bf16, kind="ExternalInput")
    wk = nc.dram_tensor("wk", [IN_DIM, 128], bf16, kind="ExternalInput")
    wv = nc.dram_tensor("wv", [IN_DIM, 128], bf16, kind="ExternalInput")
    wo0 = nc.dram_tensor("wo0", [64, OUT_DIM], bf16, kind="ExternalInput")
    wo1 = nc.dram_tensor("wo1", [64, OUT_DIM], bf16, kind="ExternalInput")
    lam = nc.dram_tensor("lam", [128, 1], f32, kind="ExternalInput")
    outT = nc.dram_tensor("outT", [OUT_DIM, T], f32, kind="ExternalOutput")

    with TileContext(nc) as tc:
        with tc.tile_pool(name="persist", bufs=1) as pp:
            # ---- persistent SBUF ----
            wq_sb = pp.tile([128, NIN, 128], bf16)
            wk_sb = pp.tile([128, NIN, 128], bf16)
            wv_sb = pp.tile([128, NIN, 128], bf16)
            wo0_sb = pp.tile([64, OUT_DIM], bf16)
            wo1_sb = pp.tile([64, OUT_DIM], bf16)
            qT_sb = pp.tile([128, T], bf16)          # 4 qk-heads x 32 rows
            kT_sb = pp.tile([128, T], bf16)
            v_sb = pp.tile([128, NKT, 130], bf16)    # [t, kt, (vA|1|vB|1)]
            lam_sb = pp.tile([128, 1], f32)
            warm = pp.tile([128, 512], bf16)

            # ---- input DMAs ----
            nc.sync.dma_start(out=lam_sb[:, :], in_=lam[:, :])
            nc.sync.dma_start(out=wq_sb[:, :, :],
                              in_=wq.rearrange("(c p) m -> p c m", p=128))
            nc.sync.dma_start(out=wk_sb[:, :, :],
                              in_=wk.rearrange("(c p) m -> p c m", p=128))
            nc.sync.dma_start(out=wv_sb[:, :, :],
                              in_=wv.rearrange("(c p) m -> p c m", p=128))

            # ---- phase A: x^T load (fine-grained), warmup, projections ----
            with tc.tile_pool(name="xpool", bufs=1) as xp:
                xT_sb = xp.tile([128, NIN, T], bf16)
                # 16 half-chunk pieces; q/k projection chases their arrival
                for c in range(NIN):
                    for hf in range(2):
                        nc.sync.dma_start(
                            out=xT_sb[:, c, hf * 1024:(hf + 1) * 1024],
                            in_=xT[128 * c:128 * (c + 1),
                                   hf * 1024:(hf + 1) * 1024])
                nc.sync.dma_start(out=wo0_sb[:, :], in_=wo0[:, :])
                nc.sync.dma_start(out=wo1_sb[:, :], in_=wo1[:, :])

                with tc.tile_pool(name="psW", bufs=1, space="PSUM") as psW:
                    nc.vector.memset(warm[:, :], 0.0)
                    wm = psW.tile([128, 512], f32, tag="warm", bufs=1)
                    for _ in range(24):
                        nc.tensor.matmul(wm[:, :], warm[:, :128], warm[:, :],
                                         start=True, stop=True)
                    nc.scalar.activation(warm[:64, :], wm[:64, :], AF.Exp)

                # q/k projections: chunk-outer so matmuls start as soon as
                # each xT piece lands; 8 concurrent PSUM accumulators
                with tc.tile_pool(name="psQK", bufs=1, space="PSUM") as psQK:
                    ps = [psQK.tile([128, QB], f32, tag=f"pj{d}{tb}",
                                    name=f"pj{d}{tb}")
                          for d in range(2) for tb in range(NQB)]
                    for c in range(NIN):
                        for d, w_sb in ((0, wq_sb), (1, wk_sb)):
                            for tb in range(NQB):
                                nc.tensor.matmul(
                                    ps[d * NQB + tb][:, :], w_sb[:, c, :],
                                    xT_sb[:, c, tb * QB:(tb + 1) * QB],
                                    start=(c == 0), stop=(c == NIN - 1))
                    for d, dst in ((0, qT_sb), (1, kT_sb)):
                        for tb in range(NQB):
                            if tb % 2 == 0:
                                nc.vector.tensor_copy(
                                    dst[:, tb * QB:(tb + 1) * QB],
                                    ps[d * NQB + tb][:, :])
                            else:
                                nc.scalar.activation(
                                    dst[:, tb * QB:(tb + 1) * QB],
                                    ps[d * NQB + tb][:, :], AF.Copy)
                with tc.tile_pool(name="psV", bufs=3, space="PSUM") as psV:
                    wmv = psV.tile([128, 512], f32, tag="wmv", bufs=1)
                    for _ in range(12):
                        nc.tensor.matmul(wmv[:, :], warm[:, :128], warm[:, :],
                                         start=True, stop=True)
                    for kt in range(NKT):
                        p = psV.tile([128, 128], f32, tag="vproj")
                        for c in range(NIN):
                            nc.tensor.matmul(
                                p[:, :], xT_sb[:, c, kt * 128:(kt + 1) * 128],
                                wv_sb[:, c, :], start=(c == 0),
                                stop=(c == NIN - 1))
                        nc.vector.tensor_copy(v_sb[:, kt, 0:64], p[:, 0:64])
                        nc.vector.tensor_copy(v_sb[:, kt, 65:129],
                                              p[:, 64:128])
                    with tc.tile_pool(name="onescr", bufs=1) as op_:
                        oscr = op_.tile([128, NKT], f32)
                        nc.vector.memset(oscr[:, :], 1.0)
                        nc.vector.tensor_copy(
                            v_sb[:, :, 64:65].rearrange("p n 1 -> p n"),
                            oscr[:, :])
                        nc.vector.tensor_copy(
                            v_sb[:, :, 129:130].rearrange("p n 1 -> p n"),
                            oscr[:, :])

            # ---- phase B: attention, with per-qb combine interleaved ----
            cmbs = {}
            ssBs = {}
            for qb in range(NQB):
                cmbs[qb] = [pp.tile([64, QB], bf16, name=f"cmb{v}_{qb}")
                            for v in range(2)]
                ssBs[qb] = [pp.tile([64, QB], f32, name=f"ssB{v}_{qb}")
                            for v in range(2)]
            with tc.tile_pool(name="cpool", bufs=2) as cp:
                def epilogue_steps(qb, accs):
                    """Combine chain for query block qb, returned as issue-
                    checkpoint -> thunks so it interleaves with the NEXT
                    block's loop (avoids FIFO head-of-line blocking: each op
                    reaches its engine only after its inputs exist)."""
                    qs = qb * QB
                    a1g = [cp.tile([65, QB], f32, tag=f"a1g{v}",
                                   name=f"a1g{v}_{qb}") for v in range(2)]
                    a2g = [cp.tile([65, QB], f32, tag=f"a2g{v}",
                                   name=f"a2g{v}_{qb}") for v in range(2)]
                    s1c = cp.tile([2, QB], f32, tag="s1c", name=f"s1c_{qb}")
                    s2c = cp.tile([2, QB], f32, tag="s2c", name=f"s2c_{qb}")
                    rec2 = cp.tile([2, QB], f32, tag="rec2", name=f"rec2_{qb}")
                    r2t = cp.tile([2, QB], f32, tag="r2t", name=f"r2t_{qb}")
                    r2b = cp.tile([1, QB], f32, tag="r2b", name=f"r2b_{qb}")
                    rb = [cp.tile([64, QB], f32, tag=f"rb{v}",
                                  name=f"rb{v}_{qb}") for v in range(2)]
                    t2 = [cp.tile([64, QB], f32, tag=f"t2{v}",
                                  name=f"t2{v}_{qb}") for v in range(2)]
                    sqq = [cp.tile([64, QB], bf16, tag=f"sq{v}",
                                   name=f"sq{v}_{qb}") for v in range(2)]
                    cmb = cmbs[qb]
                    ssB = ssBs[qb]

                    def stage():
                        for vh in range(2):
                            nc.scalar.activation(a1g[vh][:, :],
                                                 accs[2 * vh][:, :], AF.Copy)
                            nc.vector.tensor_copy(a2g[vh][:, :],
                                                  accs[2 * vh + 1][:, :])
                        for vh in range(2):
                            nc.sync.dma_start(out=s1c[vh:vh + 1, :],
                                              in_=a1g[vh][64:65, :])
                            nc.sync.dma_start(out=s2c[vh:vh + 1, :],
                                              in_=a2g[vh][64:65, :])

                    def ratio():
                        nc.vector.reciprocal_approx_fast(rec2[:, :],
                                                         s2c[:, :])
                        nc.vector.scalar_tensor_tensor(
                            r2t[:, :], s1c[:, :], lam_sb[0:2, 0:1],
                            rec2[:, :], op0=MUL, op1=MUL)
                        nc.sync.dma_start(out=r2b[0:1, :], in_=r2t[1:2, :])

                    def bcast():
                        nc.gpsimd.partition_broadcast(rb[0][:, :],
                                                      r2t[0:1, :])
                        nc.gpsimd.partition_broadcast(rb[1][:, :],
                                                      r2b[0:1, :])

                    def mul_t2():
                        for vh in range(2):
                            nc.vector.tensor_mul(t2[vh][:, :],
                                                 a2g[vh][0:64, :],
                                                 rb[vh][:, :])

                    def sub_cmb():
                        for vh in range(2):
                            nc.vector.tensor_sub(cmb[vh][:, :],
                                                 a1g[vh][0:64, :],
                                                 t2[vh][:, :])

                    def square():
                        for vh in range(2):
                            nc.scalar.activation(sqq[vh][:, :],
                                                 cmb[vh][:, :], AF.Square)

                    def allred():
                        for vh in range(2):
                            nc.gpsimd.partition_all_reduce(
                                ssB[vh][:, :], sqq[vh][:, :], 64,
                                bass_isa.ReduceOp.add)

                    return {0: stage, 2: ratio, 4: bcast, 6: mul_t2,
                            8: sub_cmb, 10: square, 12: allred}

                pend = {}
                tail = None
                with (
                    tc.tile_pool(name="dots_ps", bufs=1, space="PSUM") as dps,
                    tc.tile_pool(name="acc_ps", bufs=1, space="PSUM") as aps,
                    tc.tile_pool(name="epool", bufs=3) as ep,
                ):
                    for qb in range(NQB):
                        qs = qb * QB
                        # accs[2*vh+s]: s=0 pos, s=1 neg; 64 v dims + s row
                        accs = [aps.tile([65, QB], f32, tag=f"acc{j}",
                                         name=f"acc{j}_{qb}")
                                for j in range(4)]
                        es = {}
                        for kt in range(NKT):
                            # P gets heads 0,2 (pos); N gets heads 1,3 (neg)
                            pt = dps.tile([128, 2 * QB], f32, tag="dP",
                                          name=f"dP_{qb}_{kt}")
                            nt = dps.tile([128, 2 * QB], f32, tag="dN",
                                          name=f"dN_{qb}_{kt}")
                            for h in range(4):
                                dp = pt if h % 2 == 0 else nt
                                nc.tensor.matmul(
                                    dp[:, (h // 2) * QB:(h // 2 + 1) * QB],
                                    kT_sb[32 * h:32 * (h + 1),
                                          kt * KT:(kt + 1) * KT],
                                    qT_sb[32 * h:32 * (h + 1), qs:qs + QB],
                                    start=True, stop=True,
                                    tile_position=(32 * h, 0))
                            eP = ep.tile([128, 2 * QB], bf16, tag="eP",
                                         name=f"eP_{qb}_{kt}")
                            nc.scalar.activation(eP[:, :], pt[:, :], AF.Exp)
                            eNi = ep.tile([128, 2 * QB], i16, tag="eN",
                                          name=f"eN_{qb}_{kt}")
                            nc.vector.tensor_scalar(
                                eNi[:, :], nt[:, :], A16, B16,
                                op0=MUL, op1=ADD)
                            es[kt] = (eP, eNi.bitcast(bf16))
                            if kt > 0:
                                _attnv(nc, accs, es, v_sb, kt - 1, NKT)
                            if kt in pend:
                                pend[kt]()
                            if kt == 2:
                                # dense dummy-MM burst into the consumed dots
                                # bank: real PE activity that re-fires the
                                # HAM un-throttle (SHORT window) twice per
                                # block; next kt's start=True discards the
                                # garbage
                                for _ in range(12):
                                    nc.tensor.matmul(pt[:, 0:512],
                                                     warm[:, 0:128],
                                                     warm[:, :],
                                                     start=True, stop=True)
                            # dummy weight loads pad PE activity through the
                            # exp wait so the MID window keeps seeing work
                            for _ in range(6):
                                nc.tensor.ldweights(warm[:, 0:128])
                        _attnv(nc, accs, es, v_sb, NKT - 1, NKT)
                        pend = epilogue_steps(qb, accs)
                        if qb == NQB - 1:
                            # stage now (reads accs before the pool closes);
                            # the rest of the chain interleaves with phase C
                            pend[0]()
                            tail = [pend[k] for k in sorted(pend) if k > 0]

                # -- phase C: rms scale + out-projection, interleaved with
                # the last block's combine chain so the PE stream never
                # waits on it (no FIFO head-of-line blocking)
                with (
                    tc.tile_pool(name="psC", bufs=3, space="PSUM") as psC,
                    tc.tile_pool(name="sbC", bufs=3) as sbC,
                ):
                    def c_pre(qb):
                        finl = []
                        for vh in range(2):
                            sqr = sbC.tile([64, QB], f32, tag=f"sqr{vh}")
                            nc.scalar.activation(sqr[:, :],
                                                 ssBs[qb][vh][:, :], AF.Sqrt,
                                                 scale=1.0 / 64.0)
                            rn = sbC.tile([64, QB], f32, tag=f"rn{vh}")
                            nc.vector.reciprocal_approx_fast(rn[:, :],
                                                             sqr[:, :])
                            fl = sbC.tile([64, QB], bf16, tag=f"finl{vh}",
                                          name=f"finl{vh}_{qb}")
                            nc.vector.tensor_mul(fl[:, :],
                                                 cmbs[qb][vh][:, :],
                                                 rn[:, :])
                            finl.append(fl)
                        return finl

                    def c_proj(qb, finl, ocs):
                        qs = qb * QB
                        for oc in ocs:
                            p = psC.tile([128, QB], f32, tag="oproj")
                            nc.tensor.matmul(
                                p[:, :], wo0_sb[:, oc * 128:(oc + 1) * 128],
                                finl[0][:, :], start=True, stop=False)
                            nc.tensor.matmul(
                                p[:, :], wo1_sb[:, oc * 128:(oc + 1) * 128],
                                finl[1][:, :], start=False, stop=True)
                            o = sbC.tile([128, QB], f32, tag="ostage")
                            if oc % 2 == 0:
                                nc.vector.tensor_copy(o[:, :], p[:, :])
                            else:
                                nc.scalar.activation(o[:, :], p[:, :],
                                                     AF.Copy)
                            nc.sync.dma_start(
                                out=outT[oc * 128:(oc + 1) * 128,
                                         qs:qs + QB],
                                in_=o[:, :])
                            for _ in range(2):
                                nc.tensor.ldweights(warm[:, 0:128])

                    wmc = psC.tile([128, QB], f32, tag="oproj", bufs=3,
                                   name="wmc")
                    for _ in range(10):
                        nc.tensor.matmul(wmc[:, :], warm[:, :128], warm[:, :],
                                         start=True, stop=True)
                    f0 = c_pre(0)
                    tail[0]()                       # ratio (DVE + fan DMA)
                    c_proj(0, f0, range(4))
                    tail[1]()                       # broadcast (GP)
                    tail[2]()                       # t2 mul (DVE)
                    c_proj(0, f0, range(4, 8))
                    f1 = c_pre(1)
                    tail[3]()                       # comb sub (DVE)
                    c_proj(1, f1, range(4))
                    tail[4]()                       # square (ACT)
                    tail[5]()                       # all-reduce (GP)
                    c_proj(1, f1, range(4, 8))
                    f2 = c_pre(2)
                    wmc2 = psC.tile([128, QB], f32, tag="oproj",
                                    name="wmc2")
                    for _ in range(10):
                        nc.tensor.matmul(wmc2[:, :], warm[:, :128],
                                         warm[:, :], start=True, stop=True)
                    c_proj(2, f2, range(8))
                    f3 = c_pre(3)
                    c_proj(3, f3, range(8))

    nc.compile()
    return nc


def _attnv(nc, accs, es, v_sb, kt, nkt):
    eP, eN = es[kt]
    for j in range(4):
        e = eP if j % 2 == 0 else eN
        ecol = (j // 2) * QB
        vcol = 65 * (j // 2)
        nc.tensor.matmul(
            accs[j][:, :], v_sb[:, kt, vcol:vcol + 65],
            e[:, ecol:ecol + QB],
            start=(kt == 0), stop=(kt == nkt - 1))


def _get_compiled():
    global _compiled
    if _compiled is None:
        _compiled = _build()
    return _compiled


def make_in_maps(x, Wq, Wkv, Wout, lambda_q1, lambda_k1, lambda_q2, lambda_k2,
                 gamma):
    import ml_dtypes
    bf = ml_dtypes.bfloat16
    x = np.asarray(x, dtype=np.float32)
    Wq = np.asarray(Wq, dtype=np.float32)
    Wkv = np.asarray(Wkv, dtype=np.float32)
    Wout = np.asarray(Wout, dtype=np.float32)
    lam_v = (math.exp(float(np.dot(lambda_q1, lambda_k1)))
             - math.exp(float(np.dot(lambda_q2, lambda_k2))) + LAMBDA_INIT)
    lam_arr = np.full((128, 1), lam_v, dtype=np.float32)
    # gamma * (1 - lambda_init) folded into the out-projection rows
    gam_f = (np.asarray(gamma, dtype=np.float32)
             * (1.0 - LAMBDA_INIT)).reshape(64, 1)
    Wq_s = (Wq * (D ** -0.5)).astype(np.float32)
    Wk = Wkv[:, :E]
    Wv = Wkv[:, E:]
    xT_all = [np.ascontiguousarray(x[b].T).astype(bf) for b in range(B)]
    in_maps = []
    for c in range(N_CORES):
        b, g = divmod(c, GROUPS)
        sl = slice(128 * g, 128 * (g + 1))
        wo0 = Wout[128 * g:128 * g + 64, :] * gam_f
        wo1 = Wout[128 * g + 64:128 * (g + 1), :] * gam_f
        in_maps.append({
            "xT": xT_all[b],
            "wq": np.ascontiguousarray(Wq_s[:, sl]).astype(bf),
            "wk": np.ascontiguousarray(Wk[:, sl]).astype(bf),
            "wv": np.ascontiguousarray(Wv[:, sl]).astype(bf),
            "wo0": np.ascontiguousarray(wo0).astype(bf),
            "wo1": np.ascontiguousarray(wo1).astype(bf),
            "lam": lam_arr,
        })
    return in_maps


def kernel(x, Wq, Wkv, Wout, lambda_q1, lambda_k1, lambda_q2, lambda_k2,
           gamma, _run_kw=None):
    import sys
    if "/opt/trn_rl_repo" not in sys.path:
        sys.path.insert(0, "/opt/trn_rl_repo")
    from concourse.bass_utils import run_bass_kernel_spmd

    nc = _get_compiled()
    in_maps = make_in_maps(x, Wq, Wkv, Wout, lambda_q1, lambda_k1,
                           lambda_q2, lambda_k2, gamma)
    res = run_bass_kernel_spmd(nc, in_maps, list(range(N_CORES)),
                               **(_run_kw or {}))
    out = np.zeros((B, T, OUT_DIM), dtype=np.float32)
    for c in range(N_CORES):
        out[c // GROUPS] += res.results[c]["outT"].T
    kernel.last_result = res
    return out


# revision 28
# speedup vs baseline: 1.0501x; 1.0501x over previous
"""DiffAttn Trainium2 kernel (8-core SPMD, no collectives) — v2.

Problem: B=2, T=2048, IN_DIM=OUT_DIM=1024, H=8 v-heads (2D=64), 2H=16 qk-heads
(D=32). Core c = 4*b + g handles batch b, head-group g: qk-heads {4g..4g+3}
(local heads h=0..3; h even = positive softmax, h odd = negative), v-heads
{2g, 2g+1}. Out-projection row-sharded; host sums 4 partials per batch.

v2 structure (vs v1): the dots PSUM is split pos/neg — heads {0,2} land in P,
heads {1,3} in N. exp(P) runs on the Scalar engine (ACT); exp(N) is computed
on the Vector engine with a Schraudolph bit-trick (one tensor_scalar:
int16(A*z+B) whose bits ARE the bf16 of e^z). The sawtooth error of the
approximation only touches the negative softmax, which the end-to-end test
shows is attenuated by the lambda-weighted subtraction (~3e-3 final rel err).
This halves the ACT workload, which was the phase-B critical path, and makes
the PE dense enough to hold its high clock state (HAM).

The per-query-block combine (r = lam*s1/s2, comb = a1 - r*a2, sumsq) runs
interleaved with the next block's attention, using the otherwise-idle GpSimd
engine for partition broadcasts / reductions and elementwise work. The RMS
sqrt is batched into the tail so the ACT exp table set is never swapped
mid-attention. gamma*(1-lambda_init) is folded into Wout on the host; the
1/rms scale commutes onto comb before the out-projection.
"""
import math

import numpy as np

H = 8
D = 32
LAMBDA_INIT = 0.8 - 0.6 * math.exp(-0.3)
B, T, IN_DIM, OUT_DIM = 2, 2048, 1024, 1024
E = 2 * H * D  # 512

N_CORES = 8
GROUPS = 4            # head groups (cores per batch)
QB = 512              # query block (matmul free dim)
NQB = T // QB         # 4
KT = 128              # key tile (partition dim)
NKT = T // KT         # 16
NIN = IN_DIM // 128   # 8

# Schraudolph exp in bf16-bit-space: bf16(e^z) ~= bits(int16(A16*z + B16)).
# B16 tuned for ~zero mean rel err; constant offset cancels in softmax.
A16 = 128.0 / math.log(2.0)
B16 = 1064866805.0 / 65536.0

_compiled = None


def _build():
    import concourse.bass as bass  # noqa: F401
    import concourse.mybir as mybir
    from concourse import bacc, bass_isa
    from concourse.tile import TileContext

    f32 = mybir.dt.float32
    bf16 = mybir.dt.bfloat16
    i16 = mybir.dt.int16
    AF = mybir.ActivationFunctionType
    MUL = mybir.AluOpType.mult
    ADD = mybir.AluOpType.add

    nc = bacc.Bacc("TRN2", target_bir_lowering=False, num_devices=N_CORES)

    xT = nc.dram_tensor("xT", [IN_DIM, T], bf16, kind="ExternalInput")
    wq = nc.dram_tensor("wq", [IN_DIM, 128], bf16, kind="ExternalInput")
    wk = nc.dram_tensor("wk", [IN_DIM, 128], bf16, kind="ExternalInput")
    wv = nc.dram_tensor("wv", [IN_DIM, 128], bf16, kind="ExternalInput")
    wo0 = nc.dram_tensor("wo0", [64, OUT_DIM], bf16, kind="ExternalInput")
    wo1 = nc.dram_tensor("wo1", [64, OUT_DIM], bf16, kind="ExternalInput")
    lam = nc.dram_tensor("lam", [128, 1], f32, kind="ExternalInput")
    outT = nc.dram_tensor("outT", [OUT_DIM, T], f32, kind="ExternalOutput")

    with TileContext(nc) as tc:
        with tc.tile_pool(name="persist", bufs=1) as pp:
            # ---- persistent SBUF ----
            wq_sb = pp.tile([128, NIN, 128], bf16)
            wk_sb = pp.tile([128, NIN, 128], bf16)
            wv_sb = pp.tile([128, NIN, 128], bf16)
            wo0_sb = pp.tile([64, OUT_DIM], bf16)
            wo1_sb = pp.tile([64, OUT_DIM], bf16)
            qT_sb = pp.tile([128, T], bf16)          # 4 qk-heads x 32 rows
            kT_sb = pp.tile([128, T], bf16)
            v_sb = pp.tile([128, NKT, 130], bf16)    # [t, kt, (vA|1|vB|1)]
            lam_sb = pp.tile([128, 1], f32)
            warm = pp.tile([128, 512], bf16)

            # ---- input DMAs ----
            nc.sync.dma_start(out=lam_sb[:, :], in_=lam[:, :])
            nc.sync.dma_start(out=wq_sb[:, :, :],
                              in_=wq.rearrange("(c p) m -> p c m", p=128))
            nc.sync.dma_start(out=wk_sb[:, :, :],
                              in_=wk.rearrange("(c p) m -> p c m", p=128))
            nc.sync.dma_start(out=wv_sb[:, :, :],
                              in_=wv.rearrange("(c p) m -> p c m", p=128))

            # ---- phase A: x^T load (fine-grained), warmup, projections ----
            with tc.tile_pool(name="xpool", bufs=1) as xp:
                xT_sb = xp.tile([128, NIN, T], bf16)
                # 16 half-chunk pieces; q/k projection chases their arrival
                for c in range(NIN):
                    for hf in range(2):
                        nc.sync.dma_start(
                            out=xT_sb[:, c, hf * 1024:(hf + 1) * 1024],
                            in_=xT[128 * c:128 * (c + 1),
                                   hf * 1024:(hf + 1) * 1024])
                nc.sync.dma_start(out=wo0_sb[:, :], in_=wo0[:, :])
                nc.sync.dma_start(out=wo1_sb[:, :], in_=wo1[:, :])

                with tc.tile_pool(name="psW", bufs=1, space="PSUM") as psW:
                    nc.vector.memset(warm[:, :], 0.0)
                    wm = psW.tile([128, 512], f32, tag="warm", bufs=1)
                    for _ in range(24):
                        nc.tensor.matmul(wm[:, :], warm[:, :128], warm[:, :],
                                         start=True, stop=True)
                    nc.scalar.activation(warm[:64, :], wm[:64, :], AF.Exp)

                # q/k projections: chunk-outer so matmuls start as soon as
                # each xT piece lands; 8 concurrent PSUM accumulators
                with tc.tile_pool(name="psQK", bufs=1, space="PSUM") as psQK:
                    ps = [psQK.tile([128, QB], f32, tag=f"pj{d}{tb}",
                                    name=f"pj{d}{tb}")
                          for d in range(2) for tb in range(NQB)]
                    for c in range(NIN):
                        for d, w_sb in ((0, wq_sb), (1, wk_sb)):
                            for tb in range(NQB):
                                nc.tensor.matmul(
                                    ps[d * NQB + tb][:, :], w_sb[:, c, :],
                                    xT_sb[:, c, tb * QB:(tb + 1) * QB],
                                    start=(c == 0), stop=(c == NIN - 1))
                    for d, dst in ((0, qT_sb), (1, kT_sb)):
                        for tb in range(NQB):
                            if tb % 2 == 0:
                                nc.vector.tensor_copy(
                                    dst[:, tb * QB:(tb + 1) * QB],
                                    ps[d * NQB + tb][:, :])
                            else:
                                nc.scalar.activation(
                                    dst[:, tb * QB:(tb + 1) * QB],
                                    ps[d * NQB + tb][:, :], AF.Copy)
                with tc.tile_pool(name="psV", bufs=3, space="PSUM") as psV:
                    wmv = psV.tile([128, 512], f32, tag="wmv", bufs=1)
                    for _ in range(12):
                        nc.tensor.matmul(wmv[:, :], warm[:, :128], warm[:, :],
                                         start=True, stop=True)
                    for kt in range(NKT):
                        p = psV.tile([128, 128], f32, tag="vproj")
                        for c in range(NIN):
                            nc.tensor.matmul(
                                p[:, :], xT_sb[:, c, kt * 128:(kt + 1) * 128],
                                wv_sb[:, c, :], start=(c == 0),
                                stop=(c == NIN - 1))
                        nc.vector.tensor_copy(v_sb[:, kt, 0:64], p[:, 0:64])
                        nc.vector.tensor_copy(v_sb[:, kt, 65:129],
                                              p[:, 64:128])
                    with tc.tile_pool(name="onescr", bufs=1) as op_:
                        oscr = op_.tile([128, NKT], f32)
                        nc.vector.memset(oscr[:, :], 1.0)
                        nc.vector.tensor_copy(
                            v_sb[:, :, 64:65].rearrange("p n 1 -> p n"),
                            oscr[:, :])
                        nc.vector.tensor_copy(
                            v_sb[:, :, 129:130].rearrange("p n 1 -> p n"),
                            oscr[:, :])

            # ---- phase B: attention, with per-qb combine interleaved ----
            cmbs = {}
            ssBs = {}
            for qb in range(NQB):
                cmbs[qb] = [pp.tile([64, QB], bf16, name=f"cmb{v}_{qb}")
                            for v in range(2)]
                ssBs[qb] = [pp.tile([64, QB], f32, name=f"ssB{v}_{qb}")
                            for v in range(2)]
            with tc.tile_pool(name="cpool", bufs=2) as cp:
                def epilogue_steps(qb, accs):
                    """Combine chain for query block qb, returned as issue-
                    checkpoint -> thunks so it interleaves with the NEXT
                    block's loop (avoids FIFO head-of-line blocking: each op
                    reaches its engine only after its inputs exist)."""
                    qs = qb * QB
                    a1g = [cp.tile([65, QB], f32, tag=f"a1g{v}",
                                   name=f"a1g{v}_{qb}") for v in range(2)]
                    a2g = [cp.tile([65, QB], f32, tag=f"a2g{v}",
                                   name=f"a2g{v}_{qb}") for v in range(2)]
                    s1c = cp.tile([2, QB], f32, tag="s1c", name=f"s1c_{qb}")
                    s2c = cp.tile([2, QB], f32, tag="s2c", name=f"s2c_{qb}")
                    rec2 = cp.tile([2, QB], f32, tag="rec2", name=f"rec2_{qb}")
                    r2t = cp.tile([2, QB], f32, tag="r2t", name=f"r2t_{qb}")
                    r2b = cp.tile([1, QB], f32, tag="r2b", name=f"r2b_{qb}")
                    rb = [cp.tile([64, QB], f32, tag=f"rb{v}",
                                  name=f"rb{v}_{qb}") for v in range(2)]
                    t2 = [cp.tile([64, QB], f32, tag=f"t2{v}",
                                  name=f"t2{v}_{qb}") for v in range(2)]
                    sqq = [cp.tile([64, QB], bf16, tag=f"sq{v}",
                                   name=f"sq{v}_{qb}") for v in range(2)]
                    cmb = cmbs[qb]
                    ssB = ssBs[qb]

                    def stage():
                        for vh in range(2):
                            nc.scalar.activation(a1g[vh][:, :],
                                                 accs[2 * vh][:, :], AF.Copy)
                            nc.vector.tensor_copy(a2g[vh][:, :],
                                                  accs[2 * vh + 1][:, :])
                        for vh in range(2):
                            nc.sync.dma_start(out=s1c[vh:vh + 1, :],
                                              in_=a1g[vh][64:65, :])
                            nc.sync.dma_start(out=s2c[vh:vh + 1, :],
                                              in_=a2g[vh][64:65, :])

                    def ratio():
                        nc.vector.reciprocal_approx_fast(rec2[:, :],
                                                         s2c[:, :])
                        nc.vector.scalar_tensor_tensor(
                            r2t[:, :], s1c[:, :], lam_sb[0:2, 0:1],
                            rec2[:, :], op0=MUL, op1=MUL)
                        nc.sync.dma_start(out=r2b[0:1, :], in_=r2t[1:2, :])

                    def bcast():
                        nc.gpsimd.partition_broadcast(rb[0][:, :],
                                                      r2t[0:1, :])
                        nc.gpsimd.partition_broadcast(rb[1][:, :],
                                                      r2b[0:1, :])

                    def mul_t2():
                        for vh in range(2):
                            nc.vector.tensor_mul(t2[vh][:, :],
                                                 a2g[vh][0:64, :],
                                                 rb[vh][:, :])

                    def sub_cmb():
                        for vh in range(2):
                            nc.vector.tensor_sub(cmb[vh][:, :],
                                                 a1g[vh][0:64, :],
                                                 t2[vh][:, :])

                    def square():
                        for vh in range(2):
                            nc.scalar.activation(sqq[vh][:, :],
                                                 cmb[vh][:, :], AF.Square)

                    def allred():
                        for vh in range(2):
                            nc.gpsimd.partition_all_reduce(
                                ssB[vh][:, :], sqq[vh][:, :], 64,
                                bass_isa.ReduceOp.add)

                    return {0: stage, 2: ratio, 4: bcast, 6: mul_t2,
                            8: sub_cmb, 10: square, 12: allred}

                pend = {}
                tail = None
                with (
                    tc.tile_pool(name="dots_ps", bufs=1, space="PSUM") as dps,
                    tc.tile_pool(name="acc_ps", bufs=1, space="PSUM") as aps,
                    tc.tile_pool(name="epool", bufs=3) as ep,
                ):
                    for qb in range(NQB):
                        qs = qb * QB
                        # accs[2*vh+s]: s=0 pos, s=1 neg; 64 v dims + s row
                        accs = [aps.tile([65, QB], f32, tag=f"acc{j}",
                                         name=f"acc{j}_{qb}")
                                for j in range(4)]
                        es = {}
                        for kt in range(NKT):
                            # P gets heads 0,2 (pos); N gets heads 1,3 (neg)
                            pt = dps.tile([128, 2 * QB], f32, tag="dP",
                                          name=f"dP_{qb}_{kt}")
                            nt = dps.tile([128, 2 * QB], f32, tag="dN",
                                          name=f"dN_{qb}_{kt}")
                            for h in range(4):
                                dp = pt if h % 2 == 0 else nt
                                nc.tensor.matmul(
                                    dp[:, (h // 2) * QB:(h // 2 + 1) * QB],
                                    kT_sb[32 * h:32 * (h + 1),
                                          kt * KT:(kt + 1) * KT],
                                    qT_sb[32 * h:32 * (h + 1), qs:qs + QB],
                                    start=True, stop=True,
                                    tile_position=(32 * h, 0))
                            eP = ep.tile([128, 2 * QB], bf16, tag="eP",
                                         name=f"eP_{qb}_{kt}")
                            nc.scalar.activation(eP[:, :], pt[:, :], AF.Exp)
                            eNi = ep.tile([128, 2 * QB], i16, tag="eN",
                                          name=f"eN_{qb}_{kt}")
                            nc.vector.tensor_scalar(
                                eNi[:, :], nt[:, :], A16, B16,
                                op0=MUL, op1=ADD)
                            es[kt] = (eP, eNi.bitcast(bf16))
                            if kt > 0:
                                _attnv(nc, accs, es, v_sb, kt - 1, NKT)
                            if kt in pend:
                                pend[kt]()
                            if kt in (2, 9):
                                # dense dummy-MM burst into the consumed dots
                                # bank: real PE activity that re-fires the
                                # HAM un-throttle (SHORT window) twice per
                                # block; next kt's start=True discards the
                                # garbage
                                for _ in range(12):
                                    nc.tensor.matmul(pt[:, 0:512],
                                                     warm[:, 0:128],
                                                     warm[:, :],
                                                     start=True, stop=True)
                            # dummy weight loads pad PE activity through the
                            # exp wait so the MID window keeps seeing work
                            for _ in range(6):
                                nc.tensor.ldweights(warm[:, 0:128])
                        _attnv(nc, accs, es, v_sb, NKT - 1, NKT)
                        pend = epilogue_steps(qb, accs)
                        if qb == NQB - 1:
                            # stage now (reads accs before the pool closes);
                            # the rest of the chain interleaves with phase C
                            pend[0]()
                            tail = [pend[k] for k in sorted(pend) if k > 0]

                # -- phase C: rms scale + out-projection, interleaved with
                # the last block's combine chain so the PE stream never
                # waits on it (no FIFO head-of-line blocking)
                with (
                    tc.tile_pool(name="psC", bufs=3, space="PSUM") as psC,
                    tc.tile_pool(name="sbC", bufs=3) as sbC,
                ):
                    def c_pre(qb):
                        finl = []
                        for vh in range(2):
                            sqr = sbC.tile([64, QB], f32, tag=f"sqr{vh}")
                            nc.scalar.activation(sqr[:, :],
                                                 ssBs[qb][vh][:, :], AF.Sqrt,
                                                 scale=1.0 / 64.0)
                            rn = sbC.tile([64, QB], f32, tag=f"rn{vh}")
                            nc.vector.reciprocal_approx_fast(rn[:, :],
                                                             sqr[:, :])
                            fl = sbC.tile([64, QB], bf16, tag=f"finl{vh}",
                                          name=f"finl{vh}_{qb}")
                            nc.vector.tensor_mul(fl[:, :],
                                                 cmbs[qb][vh][:, :],
                                                 rn[:, :])
                            finl.append(fl)
                        return finl

                    def c_proj(qb, finl, ocs):
                        qs = qb * QB
                        for oc in ocs:
                            p = psC.tile([128, QB], f32, tag="oproj")
                            nc.tensor.matmul(
                                p[:, :], wo0_sb[:, oc * 128:(oc + 1) * 128],
                                finl[0][:, :], start=True, stop=False)
                            nc.tensor.matmul(
                                p[:, :], wo1_sb[:, oc * 128:(oc + 1) * 128],
                                finl[1][:, :], start=False, stop=True)
                            o = sbC.tile([128, QB], f32, tag="ostage")
                            if oc % 2 == 0:
                                nc.vector.tensor_copy(o[:, :], p[:, :])
                            else:
                                nc.scalar.activation(o[:, :], p[:, :],
                                                     AF.Copy)
                            nc.sync.dma_start(
                                out=outT[oc * 128:(oc + 1) * 128,
                                         qs:qs + QB],
                                in_=o[:, :])

                    wmc = psC.tile([128, QB], f32, tag="oproj", bufs=3,
                                   name="wmc")
                    for _ in range(10):
                        nc.tensor.matmul(wmc[:, :], warm[:, :128], warm[:, :],
                                         start=True, stop=True)
                    f0 = c_pre(0)
                    tail[0]()                       # ratio (DVE + fan DMA)
                    c_proj(0, f0, range(4))
                    tail[1]()                       # broadcast (GP)
                    tail[2]()                       # t2 mul (DVE)
                    c_proj(0, f0, range(4, 8))
                    f1 = c_pre(1)
                    tail[3]()                       # comb sub (DVE)
                    c_proj(1, f1, range(4))
                    tail[4]()                       # square (ACT)
                    tail[5]()                       # all-reduce (GP)
                    c_proj(1, f1, range(4, 8))
                    f2 = c_pre(2)
                    c_proj(2, f2, range(8))
                    f3 = c_pre(3)
                    c_proj(3, f3, range(8))

    nc.compile()
    return nc


def _attnv(nc, accs, es, v_sb, kt, nkt):
    eP, eN = es[kt]
    for j in range(4):
        e = eP if j % 2 == 0 else eN
        ecol = (j // 2) * QB
        vcol = 65 * (j // 2)
        nc.tensor.matmul(
            accs[j][:, :], v_sb[:, kt, vcol:vcol + 65],
            e[:, ecol:ecol + QB],
            start=(kt == 0), stop=(kt == nkt - 1))


def _get_compiled():
    global _compiled
    if _compiled is None:
        _compiled = _build()
    return _compiled


def make_in_maps(x, Wq, Wkv, Wout, lambda_q1, lambda_k1, lambda_q2, lambda_k2,
                 gamma):
    import ml_dtypes
    bf = ml_dtypes.bfloat16
    x = np.asarray(x, dtype=np.float32)
    Wq = np.asarray(Wq, dtype=np.float32)
    Wkv = np.asarray(Wkv, dtype=np.float32)
    Wout = np.asarray(Wout, dtype=np.float32)
    lam_v = (math.exp(float(np.dot(lambda_q1, lambda_k1)))
             - math.exp(float(np.dot(lambda_q2, lambda_k2))) + LAMBDA_INIT)
    lam_arr = np.full((128, 1), lam_v, dtype=np.float32)
    # gamma * (1 - lambda_init) folded into the out-projection rows
    gam_f = (np.asarray(gamma, dtype=np.float32)
             * (1.0 - LAMBDA_INIT)).reshape(64, 1)
    Wq_s = (Wq * (D ** -0.5)).astype(np.float32)
    Wk = Wkv[:, :E]
    Wv = Wkv[:, E:]
    xT_all = [np.ascontiguousarray(x[b].T).astype(bf) for b in range(B)]
    in_maps = []
    for c in range(N_CORES):
        b, g = divmod(c, GROUPS)
        sl = slice(128 * g, 128 * (g + 1))
        wo0 = Wout[128 * g:128 * g + 64, :] * gam_f
        wo1 = Wout[128 * g + 64:128 * (g + 1), :] * gam_f
        in_maps.append({
            "xT": xT_all[b],
            "wq": np.ascontiguousarray(Wq_s[:, sl]).astype(bf),
            "wk": np.ascontiguousarray(Wk[:, sl]).astype(bf),
            "wv": np.ascontiguousarray(Wv[:, sl]).astype(bf),
            "wo0": np.ascontiguousarray(wo0).astype(bf),
            "wo1": np.ascontiguousarray(wo1).astype(bf),
            "lam": lam_arr,
        })
    return in_maps


def kernel(x, Wq, Wkv, Wout, lambda_q1, lambda_k1, lambda_q2, lambda_k2,
           gamma, _run_kw=None):
    import sys
    if "/opt/trn_rl_repo" not in sys.path:
        sys.path.insert(0, "/opt/trn_rl_repo")
    from concourse.bass_utils import run_bass_kernel_spmd

    nc = _get_compiled()
    in_maps = make_in_maps(x, Wq, Wkv, Wout, lambda_q1, lambda_k1,
                           lambda_q2, lambda_k2, gamma)
    res = run_bass_kernel_spmd(nc, in_maps, list(range(N_CORES)),
                               **(_run_kw or {}))
    out = np.zeros((B, T, OUT_DIM), dtype=np.float32)
    for c in range(N_CORES):
        out[c // GROUPS] += res.results[c]["outT"].T
    kernel.last_result = res
    return out


# revision 33
# speedup vs baseline: 1.1032x; 1.0505x over previous
"""DiffAttn Trainium2 kernel (8-core SPMD, no collectives) — v2.

Problem: B=2, T=2048, IN_DIM=OUT_DIM=1024, H=8 v-heads (2D=64), 2H=16 qk-heads
(D=32). Core c = 4*b + g handles batch b, head-group g: qk-heads {4g..4g+3}
(local heads h=0..3; h even = positive softmax, h odd = negative), v-heads
{2g, 2g+1}. Out-projection row-sharded; host sums 4 partials per batch.

v2 structure (vs v1): the dots PSUM is split pos/neg — heads {0,2} land in P,
heads {1,3} in N. exp(P) runs on the Scalar engine (ACT); exp(N) is computed
on the Vector engine with a Schraudolph bit-trick (one tensor_scalar:
int16(A*z+B) whose bits ARE the bf16 of e^z). The sawtooth error of the
approximation only touches the negative softmax, which the end-to-end test
shows is attenuated by the lambda-weighted subtraction (~3e-3 final rel err).
This halves the ACT workload, which was the phase-B critical path, and makes
the PE dense enough to hold its high clock state (HAM).

The per-query-block combine (r = lam*s1/s2, comb = a1 - r*a2, sumsq) runs
interleaved with the next block's attention, using the otherwise-idle GpSimd
engine for partition broadcasts / reductions and elementwise work. The RMS
sqrt is batched into the tail so the ACT exp table set is never swapped
mid-attention. gamma*(1-lambda_init) is folded into Wout on the host; the
1/rms scale commutes onto comb before the out-projection.
"""
import math

import numpy as np

H = 8
D = 32
LAMBDA_INIT = 0.8 - 0.6 * math.exp(-0.3)
B, T, IN_DIM, OUT_DIM = 2, 2048, 1024, 1024
E = 2 * H * D  # 512

N_CORES = 8
GROUPS = 4            # head groups (cores per batch)
QB = 512              # query block (matmul free dim)
NQB = T // QB         # 4
KT = 128              # key tile (partition dim)
NKT = T // KT         # 16
NIN = IN_DIM // 128   # 8

# Schraudolph exp in bf16-bit-space: bf16(e^z) ~= bits(int16(A16*z + B16)).
# B16 tuned for ~zero mean rel err; constant offset cancels in softmax.
A16 = 128.0 / math.log(2.0)
B16 = 1064866805.0 / 65536.0

_compiled = None


def _build():
    import concourse.bass as bass  # noqa: F401
    import concourse.mybir as mybir
    from concourse import bacc, bass_isa
    from concourse.tile import TileContext

    f32 = mybir.dt.float32
    bf16 = mybir.dt.bfloat16
    i16 = mybir.dt.int16
    AF = mybir.ActivationFunctionType
    MUL = mybir.AluOpType.mult
    ADD = mybir.AluOpType.add

    nc = bacc.Bacc("TRN2", target_bir_lowering=False, num_devices=N_CORES)

    xT = nc.dram_tensor("xT", [IN_DIM, T], bf16, kind="ExternalInput")
    wq = nc.dram_tensor("wq", [IN_DIM, 128], bf16, kind="ExternalInput")
    wk = nc.dram_tensor("wk", [IN_DIM, 128], bf16, kind="ExternalInput")
    wv = nc.dram_tensor("wv", [IN_DIM, 128], bf16, kind="ExternalInput")
    wo0 = nc.dram_tensor("wo0", [64, OUT_DIM], bf16, kind="ExternalInput")
    wo1 = nc.dram_tensor("wo1", [64, OUT_DIM], bf16, kind="ExternalInput")
    lam = nc.dram_tensor("lam", [128, 1], f32, kind="ExternalInput")
    outT = nc.dram_tensor("outT", [OUT_DIM, T], bf16, kind="ExternalOutput")

    with TileContext(nc) as tc:
        with tc.tile_pool(name="persist", bufs=1) as pp:
            # ---- persistent SBUF ----
            wq_sb = pp.tile([128, NIN, 128], bf16)
            wk_sb = pp.tile([128, NIN, 128], bf16)
            wv_sb = pp.tile([128, NIN, 128], bf16)
            wo0_sb = pp.tile([64, OUT_DIM], bf16)
            wo1_sb = pp.tile([64, OUT_DIM], bf16)
            qT_sb = pp.tile([128, T], bf16)          # 4 qk-heads x 32 rows
            kT_sb = pp.tile([128, T], bf16)
            v_sb = pp.tile([128, NKT, 130], bf16)    # [t, kt, (vA|1|vB|1)]
            lam_sb = pp.tile([128, 1], f32)
            warm = pp.tile([128, 512], bf16)

            # ---- input DMAs ----
            nc.sync.dma_start(out=lam_sb[:, :], in_=lam[:, :])
            nc.sync.dma_start(out=wq_sb[:, :, :],
                              in_=wq.rearrange("(c p) m -> p c m", p=128))
            nc.sync.dma_start(out=wk_sb[:, :, :],
                              in_=wk.rearrange("(c p) m -> p c m", p=128))
            nc.sync.dma_start(out=wv_sb[:, :, :],
                              in_=wv.rearrange("(c p) m -> p c m", p=128))

            # ---- phase A: x^T load (fine-grained), warmup, projections ----
            with tc.tile_pool(name="xpool", bufs=1) as xp:
                xT_sb = xp.tile([128, NIN, T], bf16)
                # 16 half-chunk pieces; q/k projection chases their arrival
                for c in range(NIN):
                    for hf in range(2):
                        nc.sync.dma_start(
                            out=xT_sb[:, c, hf * 1024:(hf + 1) * 1024],
                            in_=xT[128 * c:128 * (c + 1),
                                   hf * 1024:(hf + 1) * 1024])
                nc.sync.dma_start(out=wo0_sb[:, :], in_=wo0[:, :])
                nc.sync.dma_start(out=wo1_sb[:, :], in_=wo1[:, :])

                with tc.tile_pool(name="psW", bufs=1, space="PSUM") as psW:
                    nc.vector.memset(warm[:, :], 0.0)
                    wm = psW.tile([128, 512], f32, tag="warm", bufs=1)
                    for _ in range(24):
                        nc.tensor.matmul(wm[:, :], warm[:, :128], warm[:, :],
                                         start=True, stop=True)
                    nc.scalar.activation(warm[:64, :], wm[:64, :], AF.Exp)

                # q/k projections: chunk-outer so matmuls start as soon as
                # each xT piece lands; 8 concurrent PSUM accumulators
                with tc.tile_pool(name="psQK", bufs=1, space="PSUM") as psQK:
                    ps = [psQK.tile([128, QB], f32, tag=f"pj{d}{tb}",
                                    name=f"pj{d}{tb}")
                          for d in range(2) for tb in range(NQB)]
                    for c in range(NIN):
                        for d, w_sb in ((0, wq_sb), (1, wk_sb)):
                            for tb in range(NQB):
                                nc.tensor.matmul(
                                    ps[d * NQB + tb][:, :], w_sb[:, c, :],
                                    xT_sb[:, c, tb * QB:(tb + 1) * QB],
                                    start=(c == 0), stop=(c == NIN - 1))
                    for d, dst in ((0, qT_sb), (1, kT_sb)):
                        for tb in range(NQB):
                            if tb % 2 == 0:
                                nc.vector.tensor_copy(
                                    dst[:, tb * QB:(tb + 1) * QB],
                                    ps[d * NQB + tb][:, :])
                            else:
                                nc.scalar.activation(
                                    dst[:, tb * QB:(tb + 1) * QB],
                                    ps[d * NQB + tb][:, :], AF.Copy)
                with tc.tile_pool(name="psV", bufs=3, space="PSUM") as psV:
                    wmv = psV.tile([128, 512], f32, tag="wmv", bufs=1)
                    for _ in range(12):
                        nc.tensor.matmul(wmv[:, :], warm[:, :128], warm[:, :],
                                         start=True, stop=True)
                    for kt in range(NKT):
                        p = psV.tile([128, 128], f32, tag="vproj")
                        for c in range(NIN):
                            nc.tensor.matmul(
                                p[:, :], xT_sb[:, c, kt * 128:(kt + 1) * 128],
                                wv_sb[:, c, :], start=(c == 0),
                                stop=(c == NIN - 1))
                        nc.vector.tensor_copy(v_sb[:, kt, 0:64], p[:, 0:64])
                        nc.vector.tensor_copy(v_sb[:, kt, 65:129],
                                              p[:, 64:128])
                    with tc.tile_pool(name="onescr", bufs=1) as op_:
                        oscr = op_.tile([128, NKT], f32)
                        nc.vector.memset(oscr[:, :], 1.0)
                        nc.vector.tensor_copy(
                            v_sb[:, :, 64:65].rearrange("p n 1 -> p n"),
                            oscr[:, :])
                        nc.vector.tensor_copy(
                            v_sb[:, :, 129:130].rearrange("p n 1 -> p n"),
                            oscr[:, :])

            # ---- phase B: attention, with per-qb combine interleaved ----
            cmbs = {}
            ssBs = {}
            for qb in range(NQB):
                cmbs[qb] = [pp.tile([64, QB], bf16, name=f"cmb{v}_{qb}")
                            for v in range(2)]
                ssBs[qb] = [pp.tile([64, QB], f32, name=f"ssB{v}_{qb}")
                            for v in range(2)]
            with tc.tile_pool(name="cpool", bufs=2) as cp:
                def epilogue_steps(qb, accs):
                    """Combine chain for query block qb, returned as issue-
                    checkpoint -> thunks so it interleaves with the NEXT
                    block's loop (avoids FIFO head-of-line blocking: each op
                    reaches its engine only after its inputs exist)."""
                    qs = qb * QB
                    a1g = [cp.tile([65, QB], f32, tag=f"a1g{v}",
                                   name=f"a1g{v}_{qb}") for v in range(2)]
                    a2g = [cp.tile([65, QB], f32, tag=f"a2g{v}",
                                   name=f"a2g{v}_{qb}") for v in range(2)]
                    s1c = cp.tile([2, QB], f32, tag="s1c", name=f"s1c_{qb}")
                    s2c = cp.tile([2, QB], f32, tag="s2c", name=f"s2c_{qb}")
                    rec2 = cp.tile([2, QB], f32, tag="rec2", name=f"rec2_{qb}")
                    r2t = cp.tile([2, QB], f32, tag="r2t", name=f"r2t_{qb}")
                    r2b = cp.tile([1, QB], f32, tag="r2b", name=f"r2b_{qb}")
                    rb = [cp.tile([64, QB], f32, tag=f"rb{v}",
                                  name=f"rb{v}_{qb}") for v in range(2)]
                    t2 = [cp.tile([64, QB], f32, tag=f"t2{v}",
                                  name=f"t2{v}_{qb}") for v in range(2)]
                    sqq = [cp.tile([64, QB], bf16, tag=f"sq{v}",
                                   name=f"sq{v}_{qb}") for v in range(2)]
                    cmb = cmbs[qb]
                    ssB = ssBs[qb]

                    def stage():
                        for vh in range(2):
                            nc.scalar.activation(a1g[vh][:, :],
                                                 accs[2 * vh][:, :], AF.Copy)
                            nc.vector.tensor_copy(a2g[vh][:, :],
                                                  accs[2 * vh + 1][:, :])
                        for vh in range(2):
                            nc.sync.dma_start(out=s1c[vh:vh + 1, :],
                                              in_=a1g[vh][64:65, :])
                            nc.sync.dma_start(out=s2c[vh:vh + 1, :],
                                              in_=a2g[vh][64:65, :])

                    def ratio():
                        nc.vector.reciprocal_approx_fast(rec2[:, :],
                                                         s2c[:, :])
                        nc.vector.scalar_tensor_tensor(
                            r2t[:, :], s1c[:, :], lam_sb[0:2, 0:1],
                            rec2[:, :], op0=MUL, op1=MUL)
                        nc.sync.dma_start(out=r2b[0:1, :], in_=r2t[1:2, :])

                    def bcast():
                        nc.gpsimd.partition_broadcast(rb[0][:, :],
                                                      r2t[0:1, :])
                        nc.gpsimd.partition_broadcast(rb[1][:, :],
                                                      r2b[0:1, :])

                    def mul_t2():
                        for vh in range(2):
                            nc.vector.tensor_mul(t2[vh][:, :],
                                                 a2g[vh][0:64, :],
                                                 rb[vh][:, :])

                    def sub_cmb():
                        for vh in range(2):
                            nc.vector.tensor_sub(cmb[vh][:, :],
                                                 a1g[vh][0:64, :],
                                                 t2[vh][:, :])

                    def square():
                        for vh in range(2):
                            nc.scalar.activation(sqq[vh][:, :],
                                                 cmb[vh][:, :], AF.Square)

                    def allred():
                        for vh in range(2):
                            nc.gpsimd.partition_all_reduce(
                                ssB[vh][:, :], sqq[vh][:, :], 64,
                                bass_isa.ReduceOp.add)

                    return {0: stage, 2: ratio, 4: bcast, 6: mul_t2,
                            8: sub_cmb, 10: square, 12: allred}

                pend = {}
                tail = None
                with (
                    tc.tile_pool(name="dots_ps", bufs=1, space="PSUM") as dps,
                    tc.tile_pool(name="acc_ps", bufs=1, space="PSUM") as aps,
                    tc.tile_pool(name="epool", bufs=3) as ep,
                ):
                    for qb in range(NQB):
                        qs = qb * QB
                        # accs[2*vh+s]: s=0 pos, s=1 neg; 64 v dims + s row
                        accs = [aps.tile([65, QB], f32, tag=f"acc{j}",
                                         name=f"acc{j}_{qb}")
                                for j in range(4)]
                        es = {}
                        for kt in range(NKT):
                            # P gets heads 0,2 (pos); N gets heads 1,3 (neg)
                            pt = dps.tile([128, 2 * QB], f32, tag="dP",
                                          name=f"dP_{qb}_{kt}")
                            nt = dps.tile([128, 2 * QB], f32, tag="dN",
                                          name=f"dN_{qb}_{kt}")
                            for h in range(4):
                                dp = pt if h % 2 == 0 else nt
                                nc.tensor.matmul(
                                    dp[:, (h // 2) * QB:(h // 2 + 1) * QB],
                                    kT_sb[32 * h:32 * (h + 1),
                                          kt * KT:(kt + 1) * KT],
                                    qT_sb[32 * h:32 * (h + 1), qs:qs + QB],
                                    start=True, stop=True,
                                    tile_position=(32 * h, 0))
                            eP = ep.tile([128, 2 * QB], bf16, tag="eP",
                                         name=f"eP_{qb}_{kt}")
                            nc.scalar.activation(eP[:, :], pt[:, :], AF.Exp)
                            eNi = ep.tile([128, 2 * QB], i16, tag="eN",
                                          name=f"eN_{qb}_{kt}")
                            nc.vector.tensor_scalar(
                                eNi[:, :], nt[:, :], A16, B16,
                                op0=MUL, op1=ADD)
                            es[kt] = (eP, eNi.bitcast(bf16))
                            if kt > 0:
                                _attnv(nc, accs, es, v_sb, kt - 1, NKT)
                            if kt in pend:
                                pend[kt]()
                            if kt in (2, 9):
                                # dense dummy-MM burst into the consumed dots
                                # bank: real PE activity that re-fires the
                                # HAM un-throttle (SHORT window) twice per
                                # block; next kt's start=True discards the
                                # garbage
                                for _ in range(12):
                                    nc.tensor.matmul(pt[:, 0:512],
                                                     warm[:, 0:128],
                                                     warm[:, :],
                                                     start=True, stop=True)
                            # dummy weight loads pad PE activity through the
                            # exp wait so the MID window keeps seeing work
                            for _ in range(6):
                                nc.tensor.ldweights(warm[:, 0:128])
                        _attnv(nc, accs, es, v_sb, NKT - 1, NKT)
                        pend = epilogue_steps(qb, accs)
                        if qb == NQB - 1:
                            # stage now (reads accs before the pool closes);
                            # the rest of the chain interleaves with phase C
                            pend[0]()
                            tail = [pend[k] for k in sorted(pend) if k > 0]

                # -- phase C: rms scale + out-projection, interleaved with
                # the last block's combine chain so the PE stream never
                # waits on it (no FIFO head-of-line blocking)
                with (
                    tc.tile_pool(name="psC", bufs=3, space="PSUM") as psC,
                    tc.tile_pool(name="sbC", bufs=3) as sbC,
                ):
                    def c_pre(qb):
                        finl = []
                        for vh in range(2):
                            sqr = sbC.tile([64, QB], f32, tag=f"sqr{vh}")
                            nc.scalar.activation(sqr[:, :],
                                                 ssBs[qb][vh][:, :], AF.Sqrt,
                                                 scale=1.0 / 64.0)
                            rn = sbC.tile([64, QB], f32, tag=f"rn{vh}")
                            nc.vector.reciprocal_approx_fast(rn[:, :],
                                                             sqr[:, :])
                            fl = sbC.tile([64, QB], bf16, tag=f"finl{vh}",
                                          name=f"finl{vh}_{qb}")
                            nc.vector.tensor_mul(fl[:, :],
                                                 cmbs[qb][vh][:, :],
                                                 rn[:, :])
                            finl.append(fl)
                        return finl

                    def c_proj(qb, finl, ocs):
                        qs = qb * QB
                        for oc in ocs:
                            p = psC.tile([128, QB], f32, tag="oproj",
                                         bufs=4)
                            nc.tensor.matmul(
                                p[:, :], wo0_sb[:, oc * 128:(oc + 1) * 128],
                                finl[0][:, :], start=True, stop=False)
                            nc.tensor.matmul(
                                p[:, :], wo1_sb[:, oc * 128:(oc + 1) * 128],
                                finl[1][:, :], start=False, stop=True)
                            o = sbC.tile([128, QB], bf16, tag="ostage",
                                         bufs=6)
                            if oc % 2 == 0:
                                nc.vector.tensor_copy(o[:, :], p[:, :])
                            else:
                                nc.scalar.activation(o[:, :], p[:, :],
                                                     AF.Copy)
                            nc.sync.dma_start(
                                out=outT[oc * 128:(oc + 1) * 128,
                                         qs:qs + QB],
                                in_=o[:, :])

                    wmc = psC.tile([128, QB], f32, tag="oproj", bufs=4,
                                   name="wmc")
                    for _ in range(10):
                        nc.tensor.matmul(wmc[:, :], warm[:, :128], warm[:, :],
                                         start=True, stop=True)
                    f0 = c_pre(0)
                    tail[0]()                       # ratio (DVE + fan DMA)
                    c_proj(0, f0, range(4))
                    tail[1]()                       # broadcast (GP)
                    tail[2]()                       # t2 mul (DVE)
                    c_proj(0, f0, range(4, 8))
                    f1 = c_pre(1)
                    tail[3]()                       # comb sub (DVE)
                    c_proj(1, f1, range(4))
                    tail[4]()                       # square (ACT)
                    tail[5]()                       # all-reduce (GP)
                    c_proj(1, f1, range(4, 8))
                    f2 = c_pre(2)
                    c_proj(2, f2, range(8))
                    f3 = c_pre(3)
                    c_proj(3, f3, range(8))

    nc.compile()
    return nc


def _attnv(nc, accs, es, v_sb, kt, nkt):
    eP, eN = es[kt]
    for j in range(4):
        e = eP if j % 2 == 0 else eN
        ecol = (j // 2) * QB
        vcol = 65 * (j // 2)
        nc.tensor.matmul(
            accs[j][:, :], v_sb[:, kt, vcol:vcol + 65],
            e[:, ecol:ecol + QB],
            start=(kt == 0), stop=(kt == nkt - 1))


def _get_compiled():
    global _compiled
    if _compiled is None:
        _compiled = _build()
    return _compiled


def make_in_maps(x, Wq, Wkv, Wout, lambda_q1, lambda_k1, lambda_q2, lambda_k2,
                 gamma):
    import ml_dtypes
    bf = ml_dtypes.bfloat16
    x = np.asarray(x, dtype=np.float32)
    Wq = np.asarray(Wq, dtype=np.float32)
    Wkv = np.asarray(Wkv, dtype=np.float32)
    Wout = np.asarray(Wout, dtype=np.float32)
    lam_v = (math.exp(float(np.dot(lambda_q1, lambda_k1)))
             - math.exp(float(np.dot(lambda_q2, lambda_k2))) + LAMBDA_INIT)
    lam_arr = np.full((128, 1), lam_v, dtype=np.float32)
    # gamma * (1 - lambda_init) folded into the out-projection rows
    gam_f = (np.asarray(gamma, dtype=np.float32)
             * (1.0 - LAMBDA_INIT)).reshape(64, 1)
    Wq_s = (Wq * (D ** -0.5)).astype(np.float32)
    Wk = Wkv[:, :E]
    Wv = Wkv[:, E:]
    xT_all = [np.ascontiguousarray(x[b].T).astype(bf) for b in range(B)]
    in_maps = []
    for c in range(N_CORES):
        b, g = divmod(c, GROUPS)
        sl = slice(128 * g, 128 * (g + 1))
        wo0 = Wout[128 * g:128 * g + 64, :] * gam_f
        wo1 = Wout[128 * g + 64:128 * (g + 1), :] * gam_f
        in_maps.append({
            "xT": xT_all[b],
            "wq": np.ascontiguousarray(Wq_s[:, sl]).astype(bf),
            "wk": np.ascontiguousarray(Wk[:, sl]).astype(bf),
            "wv": np.ascontiguousarray(Wv[:, sl]).astype(bf),
            "wo0": np.ascontiguousarray(wo0).astype(bf),
            "wo1": np.ascontiguousarray(wo1).astype(bf),
            "lam": lam_arr,
        })
    return in_maps


def kernel(x, Wq, Wkv, Wout, lambda_q1, lambda_k1, lambda_q2, lambda_k2,
           gamma, _run_kw=None):
    import sys
    if "/opt/trn_rl_repo" not in sys.path:
        sys.path.insert(0, "/opt/trn_rl_repo")
    from concourse.bass_utils import run_bass_kernel_spmd

    nc = _get_compiled()
    in_maps = make_in_maps(x, Wq, Wkv, Wout, lambda_q1, lambda_k1,
                           lambda_q2, lambda_k2, gamma)
    res = run_bass_kernel_spmd(nc, in_maps, list(range(N_CORES)),
                               **(_run_kw or {}))
    out = np.zeros((B, T, OUT_DIM), dtype=np.float32)
    for c in range(N_CORES):
        out[c // GROUPS] += res.results[c]["outT"].astype(np.float32).T
    kernel.last_result = res
    return out
